# revision 9
# baseline (speedup 1.0000x reference)
"""GCN (4-layer, PyG-default GCNConv) forward on 8 Trainium2 NeuronCores.

Strategy (node-parallel / graph-parallel):
  - Nodes are partitioned contiguously across the 8 cores (1250 rows each,
    padded to 1280 = 10 blocks of 128).
  - Per layer: each core computes its row-slice of G = H @ W as a tiled PE
    GEMM (bf16 in / fp32 accumulate). The slices are AllGathered into a
    replicated HBM copy in TWO row-halves (blocks 0-4 and 5-9) so the first
    collective overlaps the second half of the GEMM and the second
    collective overlaps the first half of the aggregation.
  - Aggregation (symmetric-normalized adjacency including self-loops) runs
    per 128-destination-node block: source rows are fetched with dma_gather
    (256 indices per call — this ucode's limit — round-robined over 4 SWDGE
    queues so descriptor generation parallelizes across Q7 core pairs) and
    summed on the PE as OUT_block += S_chunk.T @ MSG_chunk, where S_chunk is
    a host-built dense [128-edge, 128-dst] matrix of edge norm weights.
  - Layer 4 is reassociated as (A_hat @ H4) @ W4 + b4 so the aggregation
    stays 512 wide and the tiny final GEMM runs in fp32.
  - log_softmax over the 2 classes is fused on-chip.
"""

import sys

sys.path.insert(0, "/opt/trn_rl_repo")

import numpy as np
import ml_dtypes

BF16 = ml_dtypes.bfloat16

# Problem constants (nn_GCN_39195871543847)
N, E, F_IN, HID, C = 10000, 160000, 2208, 512, 2
W_CORES = 8
RPC = N // W_CORES  # 1250 nodes per core
MB = 10  # 128-row blocks per core
RPAD = MB * 128  # 1280
HB = MB // 2  # blocks per all-gather half
RHALF = HB * 128  # 640 rows per half
GH = W_CORES * RHALF  # 5120 rows per gathered half tensor
KFC = (F_IN + 127) // 128  # 18 contraction chunks for layer 1
KFP = KFC * 128  # 2304
C_PAD = 64  # pad 2 output classes to 64 fp32
G_CHUNKS = 2  # 128-index chunks per dma_gather call (ucode limit: 256 idxs)
N_QUEUES = 4  # SWDGE queues for gather descriptor generation


def _install_drain_patch():
    """This container's walrus accepts at most one sync-wait per instruction;
    TileContext's final drain gets one wait per live semaphore. Split the
    extra waits onto single-wait NOPs."""
    import bass_rust
    import concourse.tile as tile
    from concourse.vector_clock import ScopedClock

    if getattr(tile.TileContext, "_drain_patch_installed", False):
        return

    def _drain_and_barrier(self, tick_clock, wait_clock):
        drain_inst = self.nc.sync.drain()
        wait_clock.add_sem_waits(
            drain_inst.ins, ScopedClock({None: tick_clock.global_clock})
        )
        si = drain_inst.ins.sync_info
        waits = list(si.on_wait or []) if si is not None else []
        if len(waits) > 1:
            si.on_wait = waits[:1]
            for w in waits[1:]:
                nop = self.nc.sync.nop(nofuse=True)
                nop.ins.sync_info = bass_rust.SyncInfo(on_wait=[w], on_update=[])
        self.nc.all_engine_barrier()
        assert self.sems is not None
        popped = self.nc._tile_sem_poison_stack.pop()
        assert popped is self._sem_poison
        self.nc.clear_and_free_semaphores(list(self.sems.allocated().values()))
        self.nc.all_engine_barrier()

    tile.TileContext._drain_and_barrier = _drain_and_barrier
    tile.TileContext._drain_patch_installed = True


# ----------------------------------------------------------------------------
# Host-side graph preprocessing
# ----------------------------------------------------------------------------


def _preprocess(edge_index):
    """Per core, per 128-dst block, split incoming edges by source half and
    build the S chunk stack plus the dma_gather index layout."""
    src = edge_index[0].astype(np.int64)
    dst = edge_index[1].astype(np.int64)
    loop = np.arange(N, dtype=np.int64)
    s = np.concatenate([src, loop])
    d = np.concatenate([dst, loop])
    deg = np.bincount(d, minlength=N).astype(np.float32)
    dinv = np.where(deg > 0, 1.0 / np.sqrt(deg), 0.0).astype(np.float32)
    norm = dinv[s] * dinv[d]

    core = d // RPC
    per_core = []
    ka_fix = kb_fix = 1
    for c in range(W_CORES):
        m = core == c
        sc, dc, wc = s[m], d[m] - c * RPC, norm[m]
        # source row in the half-gathered tensors
        s_core = sc // RPC
        s_loc = sc % RPC
        in_b = s_loc >= RHALF
        g_row = s_core * RHALF + np.where(in_b, s_loc - RHALF, s_loc)
        # order: dst block, then half, stable
        blk = dc // 128
        order = np.lexsort((in_b, blk))
        sc, dc, wc = sc[order], dc[order], wc[order]
        g_row, in_b, blk = g_row[order], in_b[order], blk[order]
        mloc = dc % 128
        ca = np.bincount(blk[~in_b], minlength=MB)
        cb = np.bincount(blk[in_b], minlength=MB)
        ka_fix = max(ka_fix, int(np.max((ca + 127) // 128)))
        kb_fix = max(kb_fix, int(np.max((cb + 127) // 128)))
        per_core.append((g_row, in_b, blk, mloc, wc, ca, cb))

    TB = ka_fix + kb_fix
    T = MB * TB
    s_list, idx_list = [], []
    for g_row, in_b, blk, mloc, wc, ca, cb in per_core:
        # chunk slot for each edge: within (block, half), edges are
        # consecutive; half a occupies chunks [b*TB, b*TB+ka_fix)
        starts_a = np.zeros(MB, np.int64)
        starts_a[1:] = np.cumsum(ca)[:-1]
        starts_b = np.zeros(MB, np.int64)
        starts_b[1:] = np.cumsum(cb)[:-1]
        na = int(ca.sum())
        pos = np.empty(len(g_row), np.int64)
        ia = ~in_b
        # edges sorted by (blk, half): a-edges of blk b start at
        # starts_a[b] within the a-section etc.
        pos[ia] = np.arange(na) - starts_a[blk[ia]]
        pos[in_b] = np.arange(len(g_row) - na) - starts_b[blk[in_b]]
        t = np.where(
            in_b,
            blk * TB + ka_fix + pos // 128,
            blk * TB + pos // 128,
        )
        k = pos % 128
        S = np.zeros((T, 128, 128), np.float32)
        S[t, k, mloc] = wc
        idx_flat = np.zeros(T * 128, np.int16)
        idx_flat[t * 128 + k] = g_row.astype(np.int16)
        lay16 = idx_flat.reshape(T * 8, 16).T  # [16, T*8]
        idx_list.append(np.tile(lay16, (8, 1)).astype(np.int16))
        # SBUF-resident layout [128 partitions(k), T, 128(m)]
        s_list.append(np.ascontiguousarray(S.transpose(1, 0, 2)).astype(BF16))
    return ka_fix, kb_fix, s_list, idx_list


def _prep_inputs(x, edge_index, W1, b1, W2, b2, W3, b3, W4, b4):
    ka_fix, kb_fix, s_list, idx_list = _preprocess(edge_index)

    # xT per core: [MB, 128(p), KFC, 128(j)]; xT[m,p,k,j] = x[c*RPC+m*128+j, k*128+p]
    xts = []
    for c in range(W_CORES):
        xp = np.zeros((RPAD, KFP), np.float32)
        xp[:RPC, :F_IN] = x[c * RPC : (c + 1) * RPC]
        xt = xp.reshape(MB, 128, KFC, 128).transpose(0, 3, 2, 1)
        xts.append(np.ascontiguousarray(xt).astype(BF16))

    W1p = np.zeros((KFP, HID), np.float32)
    W1p[:F_IN] = W1
    W1l = np.ascontiguousarray(
        W1p.reshape(KFC, 128, HID).transpose(1, 0, 2)
    ).astype(BF16)
    W2l = np.ascontiguousarray(W2.reshape(4, 128, HID).transpose(1, 0, 2)).astype(BF16)
    W3l = np.ascontiguousarray(W3.reshape(4, 128, HID).transpose(1, 0, 2)).astype(BF16)
    W4p = np.zeros((HID, C_PAD), np.float32)
    W4p[:, :C] = W4
    W4l = np.ascontiguousarray(
        W4p.reshape(4, 128, C_PAD).transpose(1, 0, 2)
    ).astype(np.float32)

    b1r = np.broadcast_to(b1, (128, HID)).astype(np.float32).copy()
    b2r = np.broadcast_to(b2, (128, HID)).astype(np.float32).copy()
    b3r = np.broadcast_to(b3, (128, HID)).astype(np.float32).copy()
    b4r = np.zeros((128, C_PAD), np.float32)
    b4r[:, :C] = b4

    in_maps = []
    for c in range(W_CORES):
        in_maps.append(
            {
                "xT": xts[c],
                "W1l": W1l, "W2l": W2l, "W3l": W3l, "W4l": W4l,
                "b1r": b1r, "b2r": b2r, "b3r": b3r, "b4r": b4r,
                "S_in": s_list[c],
                "idx_in": idx_list[c],
            }
        )
    return ka_fix, kb_fix, in_maps


# ----------------------------------------------------------------------------
# Bass kernel builder
# ----------------------------------------------------------------------------

_cache = {}


def _build(ka_fix, kb_fix):
    import concourse.bass as bass
    import concourse.mybir as mybir
    from concourse.bacc import Bacc
    from concourse.tile import TileContext
    from concourse.masks import make_identity

    f32 = mybir.dt.float32
    bf16 = mybir.dt.bfloat16
    i16 = mybir.dt.int16
    TB = ka_fix + kb_fix
    T = MB * TB

    nc = Bacc(num_devices=W_CORES, num_swdge_queues=N_QUEUES)
    gq = [0]  # round-robin cursor over gather queues

    xT = nc.dram_tensor("xT", [MB, 128, KFC, 128], bf16, kind="ExternalInput")
    W1l = nc.dram_tensor("W1l", [128, KFC, HID], bf16, kind="ExternalInput")
    W2l = nc.dram_tensor("W2l", [128, 4, HID], bf16, kind="ExternalInput")
    W3l = nc.dram_tensor("W3l", [128, 4, HID], bf16, kind="ExternalInput")
    W4l = nc.dram_tensor("W4l", [128, 4, C_PAD], f32, kind="ExternalInput")
    b1r = nc.dram_tensor("b1r", [128, HID], f32, kind="ExternalInput")
    b2r = nc.dram_tensor("b2r", [128, HID], f32, kind="ExternalInput")
    b3r = nc.dram_tensor("b3r", [128, HID], f32, kind="ExternalInput")
    b4r = nc.dram_tensor("b4r", [128, C_PAD], f32, kind="ExternalInput")
    S_in = nc.dram_tensor("S_in", [128, T, 128], bf16, kind="ExternalInput")
    idx_in = nc.dram_tensor("idx_in", [128, T * 8], i16, kind="ExternalInput")
    out = nc.dram_tensor("out", [RPAD, C], f32, kind="ExternalOutput")

    # per-layer bounce halves + gathered halves (layers 0..2 are G, 3 is H4)
    own_a, own_b, full_a, full_b = [], [], [], []
    for l in range(4):
        own_a.append(nc.dram_tensor(f"own_a{l}", [RHALF, HID], bf16, kind="Internal"))
        own_b.append(nc.dram_tensor(f"own_b{l}", [RHALF, HID], bf16, kind="Internal"))
        full_a.append(
            nc.dram_tensor(
                f"full_a{l}", [GH, HID], bf16, kind="Internal", addr_space="Shared"
            )
        )
        full_b.append(
            nc.dram_tensor(
                f"full_b{l}", [GH, HID], bf16, kind="Internal", addr_space="Shared"
            )
        )

    rg = [list(range(W_CORES))]

    with TileContext(nc) as tc:
        with (
            tc.tile_pool(name="const", bufs=1) as cpool,
            tc.tile_pool(name="work", bufs=2) as wpool,
            tc.tile_pool(name="psum", bufs=2, space="PSUM") as ppool,
        ):
            relu = mybir.ActivationFunctionType.Relu

            # cached index-count registers for dma_gather
            r_full = nc.gpsimd.to_reg(G_CHUNKS * 128)
            r_half = nc.gpsimd.to_reg(128)

            # ---- layer-1 GEMM inputs first (critical path) -------------------
            W1_sb = cpool.tile([128, KFC, HID], bf16)
            nc.sync.dma_start(out=W1_sb[:], in_=W1l[:])

            def allgather(own, full):
                nc.gpsimd.collective_compute(
                    "AllGather",
                    mybir.AluOpType.bypass,
                    ins=[own[:]],
                    outs=[full[:]],
                    replica_groups=rg,
                )

            def gemm_l1():
                for m in range(MB):
                    xm = wpool.tile([128, KFC, 128], bf16, tag="xm", bufs=3)
                    nc.sync.dma_start(out=xm[:], in_=xT[m])
                    ps = ppool.tile([128, HID], f32, tag="gps")
                    for k in range(KFC):
                        nc.tensor.matmul(
                            ps[:],
                            lhsT=xm[:, k, :],
                            rhs=W1_sb[:, k, :],
                            start=(k == 0),
                            stop=(k == KFC - 1),
                        )
                    gb = wpool.tile([128, HID], bf16, tag="gb", bufs=3)
                    nc.scalar.copy(gb[:], ps[:])
                    dst = own_a[0] if m < HB else own_b[0]
                    r0 = (m % HB) * 128
                    nc.sync.dma_start(out=dst[r0 : r0 + 128, :], in_=gb[:])
                    if m == HB - 1:
                        allgather(own_a[0], full_a[0])
                allgather(own_b[0], full_b[0])

            gemm_l1()

            # ---- remaining resident tensors (overlap the first collective) ---
            S_sb = cpool.tile([128, T, 128], bf16)
            nc.sync.dma_start(out=S_sb[:], in_=S_in[:])
            idx_sb = cpool.tile([128, T * 8], i16)
            nc.sync.dma_start(out=idx_sb[:], in_=idx_in[:])
            W2_sb = cpool.tile([128, 4, HID], bf16)
            nc.sync.dma_start(out=W2_sb[:], in_=W2l[:])
            W3_sb = cpool.tile([128, 4, HID], bf16)
            nc.sync.dma_start(out=W3_sb[:], in_=W3l[:])
            W4_sb = cpool.tile([128, 4, C_PAD], f32)
            nc.sync.dma_start(out=W4_sb[:], in_=W4l[:])
            b_sb = []
            for nm, srcb in (("b1", b1r), ("b2", b2r), ("b3", b3r)):
                t = cpool.tile([128, HID], f32, tag=f"bias_{nm}")
                nc.sync.dma_start(out=t[:], in_=srcb[:])
                b_sb.append(t)
            b4_sb = cpool.tile([128, C_PAD], f32)
            nc.sync.dma_start(out=b4_sb[:], in_=b4r[:])
            id_bf = cpool.tile([128, 128], bf16)
            make_identity(nc, id_bf[:])
            id_f32 = cpool.tile([128, 128], f32)
            make_identity(nc, id_f32[:])

            def gather_chunks(ps, fsrc, t0, nk, first, last):
                """Gather nk chunks starting at chunk slot t0 from fsrc and
                accumulate S.T @ MSG into ps."""
                for g0 in range(0, nk, G_CHUNKS):
                    ngc = min(G_CHUNKS, nk - g0)
                    tg = t0 + g0
                    msg = wpool.tile([128, G_CHUNKS, HID], bf16, tag="msg", bufs=8)
                    nc.gpsimd.dma_gather(
                        out_ap=msg[:, :ngc, :],
                        in_ap=fsrc[:],
                        idxs_ap=idx_sb[:, tg * 8 : (tg + ngc) * 8],
                        num_idxs=ngc * 128,
                        num_idxs_reg=r_full if ngc == G_CHUNKS else r_half,
                        elem_size=HID,
                        queue_num=gq[0],
                    )
                    gq[0] = (gq[0] + 1) % N_QUEUES
                    for u in range(ngc):
                        nc.tensor.matmul(
                            ps[:],
                            lhsT=S_sb[:, tg + u, :],
                            rhs=msg[:, u, :],
                            start=(first and g0 == 0 and u == 0),
                            stop=(last and g0 + u == nk - 1),
                        )

            def aggregate(l, bias_t, do_relu, ht_dtype, ht_tag, ht_bufs,
                          store_h=False):
                ht_tiles = []
                for b in range(MB):
                    ps = ppool.tile([128, HID], f32, tag="aps")
                    gather_chunks(ps, full_a[l], b * TB, ka_fix, True, False)
                    gather_chunks(ps, full_b[l], b * TB + ka_fix, kb_fix, False, True)
                    if bias_t is not None:
                        hf = wpool.tile([128, HID], f32, tag="hf", bufs=3)
                        nc.vector.tensor_add(out=hf[:], in0=ps[:], in1=bias_t[:])
                        hsrc = hf
                    else:
                        hsrc = ps
                    hb = wpool.tile([128, HID], ht_dtype, tag=f"hb_{ht_tag}", bufs=3)
                    if do_relu:
                        nc.scalar.activation(hb[:], hsrc[:], relu)
                    else:
                        nc.scalar.copy(hb[:], hsrc[:])
                    if store_h:
                        dst = own_a[3] if b < HB else own_b[3]
                        r0 = (b % HB) * 128
                        nc.sync.dma_start(out=dst[r0 : r0 + 128, :], in_=hb[:])
                        if b == HB - 1:
                            allgather(own_a[3], full_a[3])
                        elif b == MB - 1:
                            allgather(own_b[3], full_b[3])
                    ident = id_bf if ht_dtype == bf16 else id_f32
                    ht = wpool.tile([128, 4, 128], ht_dtype, tag=ht_tag, bufs=ht_bufs)
                    for g in range(4):
                        tp = ppool.tile([128, 128], ht_dtype, tag="tps")
                        nc.tensor.transpose(
                            tp[:], hb[:, g * 128 : (g + 1) * 128], ident[:]
                        )
                        nc.vector.tensor_copy(out=ht[:, g, :], in_=tp[:])
                    ht_tiles.append(ht)
                return ht_tiles

            def gemm_layer(ht_tiles, w_sb, lnext):
                for m in range(MB):
                    ps = ppool.tile([128, HID], f32, tag="gps")
                    for k in range(4):
                        nc.tensor.matmul(
                            ps[:],
                            lhsT=ht_tiles[m][:, k, :],
                            rhs=w_sb[:, k, :],
                            start=(k == 0),
                            stop=(k == 3),
                        )
                    gb = wpool.tile([128, HID], bf16, tag="gb", bufs=3)
                    nc.scalar.copy(gb[:], ps[:])
                    dst = own_a[lnext] if m < HB else own_b[lnext]
                    r0 = (m % HB) * 128
                    nc.sync.dma_start(out=dst[r0 : r0 + 128, :], in_=gb[:])
                    if m == HB - 1:
                        allgather(own_a[lnext], full_a[lnext])
                    elif m == MB - 1:
                        allgather(own_b[lnext], full_b[lnext])

            # ---- layers ----------------------------------------------------
            ht2 = aggregate(0, b_sb[0], True, bf16, "ht", 12)
            gemm_layer(ht2, W2_sb, 1)
            ht3 = aggregate(1, b_sb[1], True, bf16, "ht", 12)
            gemm_layer(ht3, W3_sb, 2)
            aggregate(2, b_sb[2], True, bf16, "ht", 12, store_h=True)
            zt = aggregate(3, None, False, f32, "zt", 10)

            # ---- final GEMM + bias + log_softmax ---------------------------
            for m in range(MB):
                ps = ppool.tile([128, C_PAD], f32, tag="gps")
                for k in range(4):
                    nc.tensor.matmul(
                        ps[:],
                        lhsT=zt[m][:, k, :],
                        rhs=W4_sb[:, k, :],
                        start=(k == 0),
                        stop=(k == 3),
                    )
                lg = wpool.tile([128, C_PAD], f32, tag="lg")
                nc.vector.tensor_add(out=lg[:], in0=ps[:], in1=b4_sb[:])
                mx = wpool.tile([128, 1], f32, tag="mx")
                nc.vector.tensor_reduce(
                    out=mx[:], in_=lg[:, :C], axis=mybir.AxisListType.X,
                    op=mybir.AluOpType.max,
                )
                t2 = wpool.tile([128, C], f32, tag="t2")
                nc.vector.tensor_scalar(
                    out=t2[:], in0=lg[:, :C], scalar1=mx[:], scalar2=None,
                    op0=mybir.AluOpType.subtract,
                )
                e2 = wpool.tile([128, C], f32, tag="e2")
                nc.scalar.activation(e2[:], t2[:], mybir.ActivationFunctionType.Exp)
                sm = wpool.tile([128, 1], f32, tag="sm")
                nc.vector.tensor_reduce(
                    out=sm[:], in_=e2[:], axis=mybir.AxisListType.X,
                    op=mybir.AluOpType.add,
                )
                ls = wpool.tile([128, 1], f32, tag="ls")
                nc.scalar.activation(ls[:], sm[:], mybir.ActivationFunctionType.Ln)
                o2 = wpool.tile([128, C], f32, tag="o2")
                nc.vector.tensor_scalar(
                    out=o2[:], in0=t2[:], scalar1=ls[:], scalar2=None,
                    op0=mybir.AluOpType.subtract,
                )
                nc.sync.dma_start(out=out[m * 128 : (m + 1) * 128, :], in_=o2[:])

    nc.compile()
    return nc


# ----------------------------------------------------------------------------
# Entry point
# ----------------------------------------------------------------------------


def kernel(x, edge_index, batch, W1, b1, W2, b2, W3, b3, W4, b4, _trace=False):
    _install_drain_patch()
    from concourse.bass_utils import run_bass_kernel_spmd

    ka_fix, kb_fix, in_maps = _prep_inputs(
        np.asarray(x, np.float32),
        np.asarray(edge_index),
        np.asarray(W1, np.float32), np.asarray(b1, np.float32),
        np.asarray(W2, np.float32), np.asarray(b2, np.float32),
        np.asarray(W3, np.float32), np.asarray(b3, np.float32),
        np.asarray(W4, np.float32), np.asarray(b4, np.float32),
    )
    key = (ka_fix, kb_fix)
    if key not in _cache:
        _cache[key] = _build(ka_fix, kb_fix)
    nc = _cache[key]
    res = run_bass_kernel_spmd(
        nc, in_maps, core_ids=list(range(W_CORES)), trace=_trace
    )
    outp = np.concatenate(
        [res.results[c]["out"][:RPC] for c in range(W_CORES)], axis=0
    ).astype(np.float32)
    if _trace:
        return outp, res
    return outp


# revision 11
# speedup vs baseline: 1.0345x; 1.0345x over previous
"""GCN (4-layer, PyG-default GCNConv) forward on 8 Trainium2 NeuronCores.

Strategy (node-parallel / graph-parallel):
  - Nodes are partitioned contiguously across the 8 cores (1250 rows each,
    padded to 1280 = 10 blocks of 128).
  - Per layer: each core computes its row-slice of G = H @ W as a tiled PE
    GEMM (bf16 in / fp32 accumulate). The slices are AllGathered into a
    replicated HBM copy in TWO row-halves (blocks 0-4 and 5-9) so the first
    collective overlaps the second half of the GEMM and the second
    collective overlaps the first half of the aggregation.
  - Aggregation (symmetric-normalized adjacency including self-loops) runs
    per 128-destination-node block: source rows are fetched with dma_gather
    (256 indices per call — this ucode's limit — round-robined over 4 SWDGE
    queues so descriptor generation parallelizes across Q7 core pairs) and
    summed on the PE as OUT_block += S_chunk.T @ MSG_chunk, where S_chunk is
    a host-built dense [128-edge, 128-dst] matrix of edge norm weights.
  - Layer 4 is reassociated as (A_hat @ H4) @ W4 + b4 so the aggregation
    stays 512 wide and the tiny final GEMM runs in fp32.
  - log_softmax over the 2 classes is fused on-chip.
"""

import sys

sys.path.insert(0, "/opt/trn_rl_repo")

import numpy as np
import ml_dtypes

BF16 = ml_dtypes.bfloat16

# Problem constants (nn_GCN_39195871543847)
N, E, F_IN, HID, C = 10000, 160000, 2208, 512, 2
W_CORES = 8
RPC = N // W_CORES  # 1250 nodes per core
MB = 10  # 128-row blocks per core
RPAD = MB * 128  # 1280
HB_A = MB - 1  # blocks in the first all-gather half (0..8)
RH_A = HB_A * 128  # 1152 rows
RH_B = 128  # last block
GH_A = W_CORES * RH_A  # 9216 rows in gathered half a
GH_B = W_CORES * RH_B  # 1024 rows in gathered half b
N_WARM = 400  # low-priority PE-warming dummy matmuls
KFC = (F_IN + 127) // 128  # 18 contraction chunks for layer 1
KFP = KFC * 128  # 2304
C_PAD = 64  # pad 2 output classes to 64 fp32
G_CHUNKS = 2  # 128-index chunks per dma_gather call (ucode limit: 256 idxs)
N_QUEUES = 4  # SWDGE queues for gather descriptor generation


def _install_drain_patch():
    """This container's walrus accepts at most one sync-wait per instruction;
    TileContext's final drain gets one wait per live semaphore. Split the
    extra waits onto single-wait NOPs."""
    import bass_rust
    import concourse.tile as tile
    from concourse.vector_clock import ScopedClock

    if getattr(tile.TileContext, "_drain_patch_installed", False):
        return

    def _drain_and_barrier(self, tick_clock, wait_clock):
        drain_inst = self.nc.sync.drain()
        wait_clock.add_sem_waits(
            drain_inst.ins, ScopedClock({None: tick_clock.global_clock})
        )
        si = drain_inst.ins.sync_info
        waits = list(si.on_wait or []) if si is not None else []
        if len(waits) > 1:
            si.on_wait = waits[:1]
            for w in waits[1:]:
                nop = self.nc.sync.nop(nofuse=True)
                nop.ins.sync_info = bass_rust.SyncInfo(on_wait=[w], on_update=[])
        self.nc.all_engine_barrier()
        assert self.sems is not None
        popped = self.nc._tile_sem_poison_stack.pop()
        assert popped is self._sem_poison
        self.nc.clear_and_free_semaphores(list(self.sems.allocated().values()))
        self.nc.all_engine_barrier()

    tile.TileContext._drain_and_barrier = _drain_and_barrier
    tile.TileContext._drain_patch_installed = True


# ----------------------------------------------------------------------------
# Host-side graph preprocessing
# ----------------------------------------------------------------------------


def _preprocess(edge_index):
    """Per core, per 128-dst block, split incoming edges by source half and
    build the S chunk stack plus the dma_gather index layout."""
    src = edge_index[0].astype(np.int64)
    dst = edge_index[1].astype(np.int64)
    loop = np.arange(N, dtype=np.int64)
    s = np.concatenate([src, loop])
    d = np.concatenate([dst, loop])
    deg = np.bincount(d, minlength=N).astype(np.float32)
    dinv = np.where(deg > 0, 1.0 / np.sqrt(deg), 0.0).astype(np.float32)
    norm = dinv[s] * dinv[d]

    core = d // RPC
    per_core = []
    ka_fix = kb_fix = 1
    for c in range(W_CORES):
        m = core == c
        sc, dc, wc = s[m], d[m] - c * RPC, norm[m]
        # source row in the half-gathered tensors
        s_core = sc // RPC
        s_loc = sc % RPC
        in_b = s_loc >= RH_A
        g_row = np.where(
            in_b, s_core * RH_B + (s_loc - RH_A), s_core * RH_A + s_loc
        )
        # order: dst block, then half, stable
        blk = dc // 128
        order = np.lexsort((in_b, blk))
        sc, dc, wc = sc[order], dc[order], wc[order]
        g_row, in_b, blk = g_row[order], in_b[order], blk[order]
        mloc = dc % 128
        ca = np.bincount(blk[~in_b], minlength=MB)
        cb = np.bincount(blk[in_b], minlength=MB)
        ka_fix = max(ka_fix, int(np.max((ca + 127) // 128)))
        kb_fix = max(kb_fix, int(np.max((cb + 127) // 128)))
        per_core.append((g_row, in_b, blk, mloc, wc, ca, cb))

    TB = ka_fix + kb_fix
    T = MB * TB
    s_list, idx_list = [], []
    for g_row, in_b, blk, mloc, wc, ca, cb in per_core:
        # chunk slot for each edge: within (block, half), edges are
        # consecutive; half a occupies chunks [b*TB, b*TB+ka_fix)
        starts_a = np.zeros(MB, np.int64)
        starts_a[1:] = np.cumsum(ca)[:-1]
        starts_b = np.zeros(MB, np.int64)
        starts_b[1:] = np.cumsum(cb)[:-1]
        na = int(ca.sum())
        pos = np.empty(len(g_row), np.int64)
        ia = ~in_b
        # edges sorted by (blk, half): a-edges of blk b start at
        # starts_a[b] within the a-section etc.
        pos[ia] = np.arange(na) - starts_a[blk[ia]]
        pos[in_b] = np.arange(len(g_row) - na) - starts_b[blk[in_b]]
        t = np.where(
            in_b,
            blk * TB + ka_fix + pos // 128,
            blk * TB + pos // 128,
        )
        k = pos % 128
        S = np.zeros((T, 128, 128), np.float32)
        S[t, k, mloc] = wc
        idx_flat = np.zeros(T * 128, np.int16)
        idx_flat[t * 128 + k] = g_row.astype(np.int16)
        lay16 = idx_flat.reshape(T * 8, 16).T  # [16, T*8]
        idx_list.append(np.tile(lay16, (8, 1)).astype(np.int16))
        # SBUF-resident layout [128 partitions(k), T, 128(m)]
        s_list.append(np.ascontiguousarray(S.transpose(1, 0, 2)).astype(BF16))
    return ka_fix, kb_fix, s_list, idx_list


def _prep_inputs(x, edge_index, W1, b1, W2, b2, W3, b3, W4, b4):
    ka_fix, kb_fix, s_list, idx_list = _preprocess(edge_index)

    # xT per core: [MB, 128(p), KFC, 128(j)]; xT[m,p,k,j] = x[c*RPC+m*128+j, k*128+p]
    xts = []
    for c in range(W_CORES):
        xp = np.zeros((RPAD, KFP), np.float32)
        xp[:RPC, :F_IN] = x[c * RPC : (c + 1) * RPC]
        xt = xp.reshape(MB, 128, KFC, 128).transpose(0, 3, 2, 1)
        xts.append(np.ascontiguousarray(xt).astype(BF16))

    W1p = np.zeros((KFP, HID), np.float32)
    W1p[:F_IN] = W1
    W1l = np.ascontiguousarray(
        W1p.reshape(KFC, 128, HID).transpose(1, 0, 2)
    ).astype(BF16)
    W2l = np.ascontiguousarray(W2.reshape(4, 128, HID).transpose(1, 0, 2)).astype(BF16)
    W3l = np.ascontiguousarray(W3.reshape(4, 128, HID).transpose(1, 0, 2)).astype(BF16)
    W4p = np.zeros((HID, C_PAD), np.float32)
    W4p[:, :C] = W4
    W4l = np.ascontiguousarray(
        W4p.reshape(4, 128, C_PAD).transpose(1, 0, 2)
    ).astype(np.float32)

    b1r = np.broadcast_to(b1, (128, HID)).astype(np.float32).copy()
    b2r = np.broadcast_to(b2, (128, HID)).astype(np.float32).copy()
    b3r = np.broadcast_to(b3, (128, HID)).astype(np.float32).copy()
    b4r = np.zeros((128, C_PAD), np.float32)
    b4r[:, :C] = b4

    in_maps = []
    for c in range(W_CORES):
        in_maps.append(
            {
                "xT": xts[c],
                "W1l": W1l, "W2l": W2l, "W3l": W3l, "W4l": W4l,
                "b1r": b1r, "b2r": b2r, "b3r": b3r, "b4r": b4r,
                "S_in": s_list[c],
                "idx_in": idx_list[c],
            }
        )
    return ka_fix, kb_fix, in_maps


# ----------------------------------------------------------------------------
# Bass kernel builder
# ----------------------------------------------------------------------------

_cache = {}


def _build(ka_fix, kb_fix):
    import concourse.bass as bass
    import concourse.mybir as mybir
    from concourse.bacc import Bacc
    from concourse.tile import TileContext
    from concourse.masks import make_identity

    f32 = mybir.dt.float32
    bf16 = mybir.dt.bfloat16
    i16 = mybir.dt.int16
    TB = ka_fix + kb_fix
    T = MB * TB

    nc = Bacc(num_devices=W_CORES, num_swdge_queues=N_QUEUES)
    gq = [0]  # round-robin cursor over gather queues

    xT = nc.dram_tensor("xT", [MB, 128, KFC, 128], bf16, kind="ExternalInput")
    W1l = nc.dram_tensor("W1l", [128, KFC, HID], bf16, kind="ExternalInput")
    W2l = nc.dram_tensor("W2l", [128, 4, HID], bf16, kind="ExternalInput")
    W3l = nc.dram_tensor("W3l", [128, 4, HID], bf16, kind="ExternalInput")
    W4l = nc.dram_tensor("W4l", [128, 4, C_PAD], f32, kind="ExternalInput")
    b1r = nc.dram_tensor("b1r", [128, HID], f32, kind="ExternalInput")
    b2r = nc.dram_tensor("b2r", [128, HID], f32, kind="ExternalInput")
    b3r = nc.dram_tensor("b3r", [128, HID], f32, kind="ExternalInput")
    b4r = nc.dram_tensor("b4r", [128, C_PAD], f32, kind="ExternalInput")
    S_in = nc.dram_tensor("S_in", [128, T, 128], bf16, kind="ExternalInput")
    idx_in = nc.dram_tensor("idx_in", [128, T * 8], i16, kind="ExternalInput")
    out = nc.dram_tensor("out", [RPAD, C], f32, kind="ExternalOutput")

    # per-layer bounce halves + gathered halves (layers 0..2 are G, 3 is H4)
    own_a, own_b, full_a, full_b = [], [], [], []
    for l in range(4):
        own_a.append(nc.dram_tensor(f"own_a{l}", [RH_A, HID], bf16, kind="Internal"))
        own_b.append(nc.dram_tensor(f"own_b{l}", [RH_B, HID], bf16, kind="Internal"))
        full_a.append(
            nc.dram_tensor(
                f"full_a{l}", [GH_A, HID], bf16, kind="Internal", addr_space="Shared"
            )
        )
        full_b.append(
            nc.dram_tensor(
                f"full_b{l}", [GH_B, HID], bf16, kind="Internal", addr_space="Shared"
            )
        )

    rg = [list(range(W_CORES))]

    with TileContext(nc) as tc:
        with (
            tc.tile_pool(name="const", bufs=1) as cpool,
            tc.tile_pool(name="work", bufs=2) as wpool,
            tc.tile_pool(name="psum", bufs=2, space="PSUM") as ppool,
        ):
            relu = mybir.ActivationFunctionType.Relu

            # cached index-count registers for dma_gather
            r_full = nc.gpsimd.to_reg(G_CHUNKS * 128)
            r_half = nc.gpsimd.to_reg(128)

            # ---- layer-1 GEMM inputs first (critical path) -------------------
            W1_sb = cpool.tile([128, KFC, HID], bf16)
            nc.sync.dma_start(out=W1_sb[:], in_=W1l[:])

            def allgather(own, full):
                nc.gpsimd.collective_compute(
                    "AllGather",
                    mybir.AluOpType.bypass,
                    ins=[own[:]],
                    outs=[full[:]],
                    replica_groups=rg,
                )

            def gemm_l1():
                for m in range(MB):
                    xm = wpool.tile([128, KFC, 128], bf16, tag="xm", bufs=3)
                    nc.sync.dma_start(out=xm[:], in_=xT[m])
                    ps = ppool.tile([128, HID], f32, tag="gps")
                    for k in range(KFC):
                        nc.tensor.matmul(
                            ps[:],
                            lhsT=xm[:, k, :],
                            rhs=W1_sb[:, k, :],
                            start=(k == 0),
                            stop=(k == KFC - 1),
                        )
                    gb = wpool.tile([128, HID], bf16, tag="gb", bufs=3)
                    nc.scalar.copy(gb[:], ps[:])
                    if m < HB_A:
                        nc.sync.dma_start(
                            out=own_a[0][m * 128 : (m + 1) * 128, :], in_=gb[:]
                        )
                        if m == HB_A - 1:
                            allgather(own_a[0], full_a[0])
                    else:
                        nc.sync.dma_start(out=own_b[0][:, :], in_=gb[:])
                        allgather(own_b[0], full_b[0])

            gemm_l1()

            # ---- remaining resident tensors (overlap the first collective) ---
            S_sb = cpool.tile([128, T, 128], bf16)
            nc.sync.dma_start(out=S_sb[:], in_=S_in[:])
            idx_sb = cpool.tile([128, T * 8], i16)
            nc.sync.dma_start(out=idx_sb[:], in_=idx_in[:])
            W2_sb = cpool.tile([128, 4, HID], bf16)
            nc.sync.dma_start(out=W2_sb[:], in_=W2l[:])
            W3_sb = cpool.tile([128, 4, HID], bf16)
            nc.sync.dma_start(out=W3_sb[:], in_=W3l[:])
            W4_sb = cpool.tile([128, 4, C_PAD], f32)
            nc.sync.dma_start(out=W4_sb[:], in_=W4l[:])
            b_sb = []
            for nm, srcb in (("b1", b1r), ("b2", b2r), ("b3", b3r)):
                t = cpool.tile([128, HID], f32, tag=f"bias_{nm}")
                nc.sync.dma_start(out=t[:], in_=srcb[:])
                b_sb.append(t)
            b4_sb = cpool.tile([128, C_PAD], f32)
            nc.sync.dma_start(out=b4_sb[:], in_=b4r[:])
            id_bf = cpool.tile([128, 128], bf16)
            make_identity(nc, id_bf[:])
            id_f32 = cpool.tile([128, 128], f32)
            make_identity(nc, id_f32[:])

            def gather_chunks(ps, fsrc, t0, nk, first, last):
                """Gather nk chunks starting at chunk slot t0 from fsrc and
                accumulate S.T @ MSG into ps."""
                for g0 in range(0, nk, G_CHUNKS):
                    ngc = min(G_CHUNKS, nk - g0)
                    tg = t0 + g0
                    msg = wpool.tile([128, G_CHUNKS, HID], bf16, tag="msg", bufs=8)
                    nc.gpsimd.dma_gather(
                        out_ap=msg[:, :ngc, :],
                        in_ap=fsrc[:],
                        idxs_ap=idx_sb[:, tg * 8 : (tg + ngc) * 8],
                        num_idxs=ngc * 128,
                        num_idxs_reg=r_full if ngc == G_CHUNKS else r_half,
                        elem_size=HID,
                        queue_num=gq[0],
                    )
                    gq[0] = (gq[0] + 1) % N_QUEUES
                    for u in range(ngc):
                        nc.tensor.matmul(
                            ps[:],
                            lhsT=S_sb[:, tg + u, :],
                            rhs=msg[:, u, :],
                            start=(first and g0 == 0 and u == 0),
                            stop=(last and g0 + u == nk - 1),
                        )

            def aggregate(l, bias_t, do_relu, ht_dtype, ht_tag, ht_bufs,
                          store_h=False):
                ht_tiles = []
                for b in range(MB):
                    ps = ppool.tile([128, HID], f32, tag="aps")
                    gather_chunks(ps, full_a[l], b * TB, ka_fix, True, False)
                    gather_chunks(ps, full_b[l], b * TB + ka_fix, kb_fix, False, True)
                    if bias_t is not None:
                        hf = wpool.tile([128, HID], f32, tag="hf", bufs=3)
                        nc.vector.tensor_add(out=hf[:], in0=ps[:], in1=bias_t[:])
                        hsrc = hf
                    else:
                        hsrc = ps
                    hb = wpool.tile([128, HID], ht_dtype, tag=f"hb_{ht_tag}", bufs=3)
                    if do_relu:
                        nc.scalar.activation(hb[:], hsrc[:], relu)
                    else:
                        nc.scalar.copy(hb[:], hsrc[:])
                    if store_h:
                        if b < HB_A:
                            nc.sync.dma_start(
                                out=own_a[3][b * 128 : (b + 1) * 128, :], in_=hb[:]
                            )
                            if b == HB_A - 1:
                                allgather(own_a[3], full_a[3])
                        else:
                            nc.sync.dma_start(out=own_b[3][:, :], in_=hb[:])
                            allgather(own_b[3], full_b[3])
                    ident = id_bf if ht_dtype == bf16 else id_f32
                    ht = wpool.tile([128, 4, 128], ht_dtype, tag=ht_tag, bufs=ht_bufs)
                    for g in range(4):
                        tp = ppool.tile([128, 128], ht_dtype, tag="tps")
                        nc.tensor.transpose(
                            tp[:], hb[:, g * 128 : (g + 1) * 128], ident[:]
                        )
                        nc.vector.tensor_copy(out=ht[:, g, :], in_=tp[:])
                    ht_tiles.append(ht)
                return ht_tiles

            def gemm_layer(ht_tiles, w_sb, lnext):
                for m in range(MB):
                    ps = ppool.tile([128, HID], f32, tag="gps")
                    for k in range(4):
                        nc.tensor.matmul(
                            ps[:],
                            lhsT=ht_tiles[m][:, k, :],
                            rhs=w_sb[:, k, :],
                            start=(k == 0),
                            stop=(k == 3),
                        )
                    gb = wpool.tile([128, HID], bf16, tag="gb", bufs=3)
                    nc.scalar.copy(gb[:], ps[:])
                    if m < HB_A:
                        nc.sync.dma_start(
                            out=own_a[lnext][m * 128 : (m + 1) * 128, :], in_=gb[:]
                        )
                        if m == HB_A - 1:
                            allgather(own_a[lnext], full_a[lnext])
                    else:
                        nc.sync.dma_start(out=own_b[lnext][:, :], in_=gb[:])
                        allgather(own_b[lnext], full_b[lnext])

            # ---- layers ----------------------------------------------------
            ht2 = aggregate(0, b_sb[0], True, bf16, "ht", 12)
            gemm_layer(ht2, W2_sb, 1)
            ht3 = aggregate(1, b_sb[1], True, bf16, "ht", 12)
            gemm_layer(ht3, W3_sb, 2)
            aggregate(2, b_sb[2], True, bf16, "ht", 12, store_h=True)
            zt = aggregate(3, None, False, f32, "zt", 10)

            # ---- PE-warming filler: low-priority tiny matmuls that the
            # scheduler drops into PE-idle gaps so the HAM clock gate stays
            # at 8/8. Serialized by WAW on one PSUM tile.
            warm_ps = ppool.tile([128, 64], f32, tag="warm", bufs=1)
            for i in range(N_WARM):
                mm = nc.tensor.matmul(
                    warm_ps[:],
                    lhsT=id_bf[:],
                    rhs=id_bf[:, :64],
                    start=True,
                    stop=True,
                )
                mm.ins.bass_priority = 10_000_000 + i

            # ---- final GEMM + bias + log_softmax ---------------------------
            for m in range(MB):
                ps = ppool.tile([128, C_PAD], f32, tag="gps")
                for k in range(4):
                    nc.tensor.matmul(
                        ps[:],
                        lhsT=zt[m][:, k, :],
                        rhs=W4_sb[:, k, :],
                        start=(k == 0),
                        stop=(k == 3),
                    )
                lg = wpool.tile([128, C_PAD], f32, tag="lg")
                nc.vector.tensor_add(out=lg[:], in0=ps[:], in1=b4_sb[:])
                mx = wpool.tile([128, 1], f32, tag="mx")
                nc.vector.tensor_reduce(
                    out=mx[:], in_=lg[:, :C], axis=mybir.AxisListType.X,
                    op=mybir.AluOpType.max,
                )
                t2 = wpool.tile([128, C], f32, tag="t2")
                nc.vector.tensor_scalar(
                    out=t2[:], in0=lg[:, :C], scalar1=mx[:], scalar2=None,
                    op0=mybir.AluOpType.subtract,
                )
                e2 = wpool.tile([128, C], f32, tag="e2")
                nc.scalar.activation(e2[:], t2[:], mybir.ActivationFunctionType.Exp)
                sm = wpool.tile([128, 1], f32, tag="sm")
                nc.vector.tensor_reduce(
                    out=sm[:], in_=e2[:], axis=mybir.AxisListType.X,
                    op=mybir.AluOpType.add,
                )
                ls = wpool.tile([128, 1], f32, tag="ls")
                nc.scalar.activation(ls[:], sm[:], mybir.ActivationFunctionType.Ln)
                o2 = wpool.tile([128, C], f32, tag="o2")
                nc.vector.tensor_scalar(
                    out=o2[:], in0=t2[:], scalar1=ls[:], scalar2=None,
                    op0=mybir.AluOpType.subtract,
                )
                nc.sync.dma_start(out=out[m * 128 : (m + 1) * 128, :], in_=o2[:])

    nc.compile()
    return nc


# ----------------------------------------------------------------------------
# Entry point
# ----------------------------------------------------------------------------


def kernel(x, edge_index, batch, W1, b1, W2, b2, W3, b3, W4, b4, _trace=False):
    _install_drain_patch()
    from concourse.bass_utils import run_bass_kernel_spmd

    ka_fix, kb_fix, in_maps = _prep_inputs(
        np.asarray(x, np.float32),
        np.asarray(edge_index),
        np.asarray(W1, np.float32), np.asarray(b1, np.float32),
        np.asarray(W2, np.float32), np.asarray(b2, np.float32),
        np.asarray(W3, np.float32), np.asarray(b3, np.float32),
        np.asarray(W4, np.float32), np.asarray(b4, np.float32),
    )
    key = (ka_fix, kb_fix)
    if key not in _cache:
        _cache[key] = _build(ka_fix, kb_fix)
    nc = _cache[key]
    res = run_bass_kernel_spmd(
        nc, in_maps, core_ids=list(range(W_CORES)), trace=_trace
    )
    outp = np.concatenate(
        [res.results[c]["out"][:RPC] for c in range(W_CORES)], axis=0
    ).astype(np.float32)
    if _trace:
        return outp, res
    return outp
